# revision 1
# baseline (speedup 1.0000x reference)
"""CrossProductLayer kernel for Trainium2 (Bass/Tile), 8-core data parallel.

out[b, :] = concat(x[b]**2, x[b], 0.5 * x[b,i]*x[b,j] for i<j) * w

Full inputs:  x [16384, 128] f32, w [8384] f32.
Full output:  [16384, 8384] f32.

Sharding: pure data parallel on the batch dim — each of the 8 cores gets
2048 rows of x; w (pre-scaled and pre-broadcast to [128, 8384] on host) is
replicated. No collectives needed (forward only).

Per-core device kernel (16 row-tiles of 128 batch rows):
  - squares  -> ScalarE (Square activation)
  - singles  -> DMA'd straight from HBM into the output tile
  - pairs    -> per-i blocks out[:, blk_i] = x[:, i] * x[:, i+1:]:
               wide blocks (i < K_ACT) on ScalarE via activation scale,
               the rest on VectorE tensor_scalar (fp32 2x mode; odd widths
               padded by one column which the next block overwrites)
  - *w pass  -> one full-width VectorE tensor_tensor multiply
  - store    -> one 4.3 MB HWDGE DMA per tile
"""

import numpy as np

B = 16384
NI = 128
NF = NI + NI + (NI * (NI - 1)) // 2  # 8384
NCORES = 8
ROWS = B // NCORES  # 2048
TILE_P = 128
TILES = ROWS // TILE_P  # 16
PAIRS_OFF = 2 * NI  # 256
K_ACT = 53  # pair blocks 0..K_ACT-1 run on ScalarE, the rest on VectorE

_CACHE = {}


def _build_nc():
    import os

    # precise (unbounded) overlap tracking: the padded TS blocks and the
    # half-tile *w passes need byte-range-accurate deps, not the
    # conservative fallback past 100 pairwise checks
    os.environ["TILE_EXHAUSTIVE_MEMORY_SHARE_CHECK"] = "1"
    from concourse import bacc
    import concourse.mybir as mybir
    from concourse.tile import TileContext

    f32 = mybir.dt.float32
    nc = bacc.Bacc(
        "TRN2",
        target_bir_lowering=False,
        debug=False,
        num_devices=NCORES,
    )
    x_d = nc.dram_tensor("x", [ROWS, NI], f32, kind="ExternalInput")
    w_d = nc.dram_tensor("w", [NI, NF], f32, kind="ExternalInput")
    o_d = nc.dram_tensor("out", [ROWS, NF], f32, kind="ExternalOutput")

    with TileContext(nc) as tc:
        with (
            tc.tile_pool(name="wp", bufs=1) as wp,
            tc.tile_pool(name="xp", bufs=4) as xp,
            tc.tile_pool(name="op", bufs=4) as op,
        ):
            w_t = wp.tile([NI, NF], f32)
            nc.sync.dma_start(out=w_t[:], in_=w_d[:])
            for t in range(TILES):
                r0 = t * TILE_P
                x_t = xp.tile([TILE_P, NI + 2], f32)
                nc.sync.dma_start(out=x_t[:, 0:NI], in_=x_d[r0 : r0 + TILE_P])
                # output tile; 16 spare cols so the last padded pair block
                # can spill one column past NF
                o_t = op.tile([TILE_P, NF + 16], f32)
                # singles block [NI:2*NI) comes straight from HBM
                nc.sync.dma_start(out=o_t[:, NI : 2 * NI], in_=x_d[r0 : r0 + TILE_P])
                # squares block [0:NI)
                nc.scalar.square(o_t[:, 0:NI], x_t[:, 0:NI])
                off = PAIRS_OFF
                for i in range(NI - 1):
                    wdt = NI - 1 - i
                    sc = x_t[:, i : i + 1]
                    if i < K_ACT:
                        nc.scalar.mul(
                            o_t[:, off : off + wdt], x_t[:, i + 1 : i + 1 + wdt], sc
                        )
                    else:
                        # pad odd widths to even for the DVE fp32 2x mode;
                        # the padded column is overwritten by block i+1
                        wpad = wdt + (wdt & 1)
                        nc.vector.tensor_scalar_mul(
                            o_t[:, off : off + wpad],
                            x_t[:, i + 1 : i + 1 + wpad],
                            sc,
                        )
                    off += wdt
                # the *w pass and store in two halves: the first half's
                # store can start while the second half is still being
                # multiplied (16.8 KB HBM rows stay at full DMA rate)
                H = NF // 2
                nc.vector.tensor_mul(o_t[:, 0:H], o_t[:, 0:H], w_t[:, 0:H])
                nc.sync.dma_start(
                    out=o_d[r0 : r0 + TILE_P, 0:H], in_=o_t[:, 0:H]
                )
                nc.vector.tensor_mul(o_t[:, H:NF], o_t[:, H:NF], w_t[:, H:NF])
                nc.sync.dma_start(
                    out=o_d[r0 : r0 + TILE_P, H:NF], in_=o_t[:, H:NF]
                )
    nc.compile()
    return nc


def _get_nc():
    if "nc" not in _CACHE:
        _CACHE["nc"] = _build_nc()
    return _CACHE["nc"]


def _prep_in_maps(x, w):
    x = np.ascontiguousarray(np.asarray(x, dtype=np.float32))
    w = np.asarray(w, dtype=np.float32)
    w_scaled = w.copy()
    w_scaled[PAIRS_OFF:] *= np.float32(0.5)
    w_b = np.ascontiguousarray(np.broadcast_to(w_scaled[None, :], (NI, NF)))
    return [
        {"x": np.ascontiguousarray(x[c * ROWS : (c + 1) * ROWS]), "w": w_b}
        for c in range(NCORES)
    ]


def _run(x, w, trace=False, tmpdir=None):
    from concourse.bass_utils import run_bass_kernel_spmd

    nc = _get_nc()
    in_maps = _prep_in_maps(x, w)
    res = run_bass_kernel_spmd(
        nc, in_maps, list(range(NCORES)), trace=trace, tmpdir=tmpdir
    )
    out = np.concatenate([res.results[c]["out"] for c in range(NCORES)], axis=0)
    return out, res


def kernel(**inputs):
    out, _ = _run(inputs["x"], inputs["w"])
    return out

